# revision 12
# baseline (speedup 1.0000x reference)
"""Trainium2 Bass kernel for CategoricalEntropyRegLoss.

Math: both loss terms factor so the [B,B] pairwise matrices are never built.

  feat_dists = sq_j + sq_k - 2 fn_j.fn_k            (rank FD+2)
  target_dists = (E_j - P_j.LQ_k) / D               (rank DC+1)
  S = sum_{jk} m_j m_k feat_dists * target_dists    (diag is exactly 0)
    = [ se*M + a*e - 2 Fe.F - Psq.L - Pbar.Lsq + 2 <U,V> ] / D
  tightness*M = a - sum_s ||seg_sum_s||^2 / max(cnt_s,1)

Everything needed is one matmul per core:
  out[1154, 258] = ext_seg^T @ ext_feat
  ext_seg  = [ onehot(code) | LQ | P | 1 | E ]      (B x 1154)
  ext_feat = [ m*fn | m | m*sq ]                    (B x 258)
followed by a single 8-core AllReduce of the [1154,258] partials and a
cheap redundant epilogue on every core.
"""

import numpy as np

B = 4096
FD = 256
C = 32
D = 2
NSEG = C ** D          # 1024
NCORES = 8
RB = B // NCORES       # 512 rows per core
KT = RB // 128         # 4 k-chunks of 128 rows
EF = FD + 2            # 258: [mfn | m | m*sq]
ES = NSEG + 2 * D * C + 2   # 1154: [onehot | LQ | P | ones | E]
PCOL = NSEG + D * C    # 1088: start of P block
LCOL = NSEG            # 1024: start of LQ block
ONES_COL = NSEG + 2 * D * C      # 1152
E_COL = ONES_COL + 1             # 1153
NMT = (ES + 127) // 128          # 10 m-tiles (last has 2 rows)

_compiled = {}


def _build_bass():
    from contextlib import ExitStack
    import concourse.bass as bass
    import concourse.bacc as bacc
    import concourse.tile as tile
    from concourse import mybir

    f32 = mybir.dt.float32
    Alu = mybir.AluOpType
    Act = mybir.ActivationFunctionType
    Ax = mybir.AxisListType

    nc = bacc.Bacc(num_devices=NCORES)

    feat = nc.dram_tensor("features", [RB, FD], f32, kind="ExternalInput")
    targ = nc.dram_tensor("targets", [RB, D * C], f32, kind="ExternalInput")
    maskf = nc.dram_tensor("maskf", [RB, 1], f32, kind="ExternalInput")
    outd = nc.dram_tensor("out", [8], f32, kind="ExternalOutput")

    with ExitStack() as ctx:
        tc = ctx.enter_context(tile.TileContext(nc))
        consts = ctx.enter_context(tc.tile_pool(name="consts", bufs=1))
        io = ctx.enter_context(tc.tile_pool(name="io", bufs=3))
        work = ctx.enter_context(tc.tile_pool(name="work", bufs=2))
        keep = ctx.enter_context(tc.tile_pool(name="keep", bufs=1))
        res_pool = ctx.enter_context(tc.tile_pool(name="res", bufs=3))
        psum = ctx.enter_context(tc.tile_pool(name="psum", bufs=2, space="PSUM"))
        dram = ctx.enter_context(tc.tile_pool(name="dram", bufs=1, space="DRAM"))

        # ---------------- constants ----------------
        # DVE-owned copies so const reads never carry cross-engine waits
        # (the STT/TensorScalarPtr encodings have few sync-wait slots).
        iota_g = consts.tile([128, NSEG], f32)
        nc.gpsimd.iota(iota_g[:], [[1, NSEG]], channel_multiplier=0,
                       allow_small_or_imprecise_dtypes=True)
        iota1024 = consts.tile([128, NSEG], f32)
        nc.vector.tensor_copy(out=iota1024[:], in_=iota_g[:])
        # biota[j] = 32 - j  (for first-argmax via reduce_max)
        biota_g = consts.tile([128, C], f32)
        nc.gpsimd.iota(biota_g[:], [[-1, C]], base=C, channel_multiplier=0,
                       allow_small_or_imprecise_dtypes=True)
        biota = consts.tile([128, C], f32)
        nc.vector.tensor_copy(out=biota[:], in_=biota_g[:])
        ones128 = consts.tile([128, 1], f32)
        nc.vector.memset(ones128[:], 1.0)

        inbounce = dram.tile([ES, EF], f32, name="inbounce")
        outbounce = dram.tile([ES, EF], f32, name="outbounce", addr_space="Shared")

        es_tiles = []
        ef_tiles = []
        for kc in range(KT):
            sl = slice(kc * 128, (kc + 1) * 128)
            x = io.tile([128, FD], f32, name=f"x_{kc}", tag="x")
            nc.sync.dma_start(out=x[:], in_=feat[sl, :])
            t = io.tile([128, D * C], f32, name=f"t_{kc}", tag="t")
            nc.sync.dma_start(out=t[:], in_=targ[sl, :])
            mk = io.tile([128, 1], f32, name=f"mk_{kc}", tag="mk")
            nc.sync.dma_start(out=mk[:], in_=maskf[sl, :])

            ef_t = keep.tile([128, EF], f32, name=f"ef_{kc}")
            es_t = keep.tile([128, ES], f32, name=f"es_{kc}")

            # ---- feature normalize ----
            scr = work.tile([128, FD], f32, name=f"scr_{kc}", tag="scr")
            sq0 = work.tile([128, 1], f32, name=f"sq0_{kc}", tag="sq0")
            nc.scalar.activation(out=scr[:], in_=x[:], func=Act.Square,
                                 accum_out=sq0[:])
            norm = work.tile([128, 1], f32, name=f"norm_{kc}", tag="norm")
            nc.scalar.sqrt(norm[:], sq0[:])
            nc.vector.tensor_scalar_max(out=norm[:], in0=norm[:], scalar1=1e-12)
            inv = work.tile([128, 1], f32, name=f"inv_{kc}", tag="inv")
            nc.vector.reciprocal(inv[:], norm[:])
            fn = work.tile([128, FD], f32, name=f"fn_{kc}", tag="fn")
            nc.vector.tensor_scalar_mul(out=fn[:], in0=x[:], scalar1=inv[:])
            sq = work.tile([128, 1], f32, name=f"sq_{kc}", tag="sq")
            nc.scalar.activation(out=scr[:], in_=fn[:], func=Act.Square,
                                 accum_out=sq[:])
            # ext_feat = [m*fn | m | m*sq]
            nc.vector.tensor_scalar_mul(out=ef_t[:, 0:FD], in0=fn[:], scalar1=mk[:])
            nc.vector.tensor_copy(out=ef_t[:, FD:FD + 1], in_=mk[:])
            nc.vector.tensor_tensor(out=ef_t[:, FD + 1:FD + 2], in0=mk[:],
                                    in1=sq[:], op=Alu.mult)

            # ---- target probs / logs / entropy ----
            t1 = work.tile([128, D * C], f32, name=f"t1_{kc}", tag="t1")
            nc.vector.tensor_scalar_add(out=t1[:], in0=t[:], scalar1=1e-10)
            ssum = work.tile([128, D], f32, name=f"ssum_{kc}", tag="ssum")
            nc.vector.reduce_sum(out=ssum[:],
                                 in_=t1[:].rearrange("p (d c) -> p d c", c=C),
                                 axis=Ax.X)
            invs = work.tile([128, D], f32, name=f"invs_{kc}", tag="invs")
            nc.vector.reciprocal(invs[:], ssum[:])
            for d_ in range(D):
                nc.vector.tensor_scalar_mul(
                    out=es_t[:, PCOL + C * d_:PCOL + C * (d_ + 1)],
                    in0=t1[:, C * d_:C * (d_ + 1)],
                    scalar1=invs[:, d_:d_ + 1])
            nc.scalar.activation(out=es_t[:, LCOL:LCOL + D * C],
                                 in_=es_t[:, PCOL:PCOL + D * C], func=Act.Ln)
            scr64 = work.tile([128, D * C], f32, name=f"scr64_{kc}", tag="scr64")
            nc.vector.tensor_tensor(
                out=scr64[:], in0=es_t[:, PCOL:PCOL + D * C],
                in1=es_t[:, LCOL:LCOL + D * C], op=Alu.mult)
            nc.vector.reduce_sum(out=es_t[:, E_COL:E_COL + 1], in_=scr64[:],
                                 axis=Ax.X)
            nc.vector.memset(es_t[:, ONES_COL:ONES_COL + 1], 1.0)

            # ---- first-argmax per dim, then code = cls0 + 32*cls1 ----
            cls = work.tile([128, D], f32, name=f"cls_{kc}", tag="cls")
            for d_ in range(D):
                pch = es_t[:, PCOL + C * d_:PCOL + C * (d_ + 1)]
                mx = work.tile([128, 1], f32, name=f"mx_{kc}_{d_}", tag="mx")
                nc.vector.reduce_max(out=mx[:], in_=pch, axis=Ax.X)
                cand = work.tile([128, C], f32, name=f"cand_{kc}_{d_}", tag="cand")
                # (p == max) * (32 - idx); reduce_max -> 32 - first_argmax
                nc.vector.scalar_tensor_tensor(
                    out=cand[:], in0=pch, scalar=mx[:], in1=biota[:],
                    op0=Alu.is_equal, op1=Alu.mult)
                mq = work.tile([128, 1], f32, name=f"mq_{kc}_{d_}", tag="mq")
                nc.vector.reduce_max(out=mq[:], in_=cand[:], axis=Ax.X)
                nc.vector.tensor_scalar(
                    out=cls[:, d_:d_ + 1], in0=mq[:], scalar1=-1.0,
                    scalar2=float(C), op0=Alu.mult, op1=Alu.add)
            code = work.tile([128, 1], f32, name=f"code_{kc}", tag="code")
            nc.vector.tensor_scalar(
                out=code[:], in0=cls[:, 1:2], scalar1=float(C),
                scalar2=cls[:, 0:1], op0=Alu.mult, op1=Alu.add)
            # ---- one-hot ----
            nc.vector.tensor_scalar(
                out=es_t[:, 0:NSEG], in0=iota1024[:], scalar1=code[:],
                scalar2=None, op0=Alu.is_equal)

            es_tiles.append(es_t)
            ef_tiles.append(ef_t)

        # ---------------- the one big matmul ----------------
        for mt in range(NMT):
            mlo = mt * 128
            msz = min(128, ES - mlo)
            ps = psum.tile([msz, EF], f32, name=f"ps_{mt}", tag="ps")
            for kc in range(KT):
                nc.tensor.matmul(
                    out=ps[:], lhsT=es_tiles[kc][:, mlo:mlo + msz],
                    rhs=ef_tiles[kc][:], start=(kc == 0), stop=(kc == KT - 1))
            rt = res_pool.tile([msz, EF], f32, name=f"rt_{mt}", tag="rt")
            nc.vector.tensor_copy(out=rt[:], in_=ps[:])
            nc.sync.dma_start(out=inbounce[mlo:mlo + msz, :], in_=rt[:])

        # ---------------- single AllReduce ----------------
        nc.gpsimd.collective_compute(
            "AllReduce", mybir.AluOpType.add,
            replica_groups=[list(range(NCORES))],
            ins=[inbounce.opt()], outs=[outbounce.opt()])

        # ---------------- epilogue (redundant on every core) ----------------
        Z = keep.tile([128, 8], f32, name="Z")
        nc.vector.memset(Z[:], 0.0)
        acc = keep.tile([128, 1], f32, name="acc")
        nc.vector.memset(acc[:], 0.0)
        for st in range(NSEG // 128):
            sgt = work.tile([128, EF], f32, name=f"sgt_{st}", tag="sgt", bufs=3)
            nc.sync.dma_start(out=sgt[:],
                              in_=outbounce[st * 128:(st + 1) * 128, :])
            scrP = work.tile([128, FD], f32, name=f"scrP_{st}", tag="scrP")
            nrm = work.tile([128, 1], f32, name=f"nrm_{st}", tag="nrm")
            nc.scalar.activation(out=scrP[:], in_=sgt[:, 0:FD],
                                 func=Act.Square, accum_out=nrm[:])
            cd = work.tile([128, 1], f32, name=f"cd_{st}", tag="cd")
            nc.vector.tensor_scalar_max(out=cd[:], in0=sgt[:, FD:FD + 1],
                                        scalar1=1.0)
            rcd = work.tile([128, 1], f32, name=f"rcd_{st}", tag="rcd")
            nc.vector.reciprocal(rcd[:], cd[:])
            term = work.tile([128, 1], f32, name=f"term_{st}", tag="term")
            nc.vector.tensor_tensor(out=term[:], in0=nrm[:], in1=rcd[:],
                                    op=Alu.mult)
            nc.vector.tensor_tensor(out=acc[:], in0=acc[:], in1=term[:],
                                    op=Alu.add)
        nc.vector.tensor_copy(out=Z[:, 0:1], in_=acc[:])

        ut = keep.tile([64, EF], f32, name="ut")
        nc.sync.dma_start(out=ut[:], in_=outbounce[LCOL:LCOL + D * C, :])
        vt = keep.tile([64, EF], f32, name="vt")
        nc.sync.dma_start(out=vt[:], in_=outbounce[PCOL:PCOL + D * C, :])
        last2 = keep.tile([1, EF], f32, name="last2")
        nc.sync.dma_start(out=last2[:], in_=outbounce[ONES_COL:ONES_COL + 1, :])
        r1 = keep.tile([1, EF], f32, name="r1")
        nc.sync.dma_start(out=r1[:], in_=outbounce[E_COL:E_COL + 1, :])

        scrU = keep.tile([64, FD], f32, name="scrU")
        nc.vector.tensor_tensor(out=scrU[:], in0=ut[:, 0:FD], in1=vt[:, 0:FD],
                                op=Alu.mult)
        nc.vector.reduce_sum(out=Z[0:64, 1:2], in_=scrU[:], axis=Ax.X)
        nc.vector.tensor_tensor(out=Z[0:64, 2:3], in0=vt[:, FD + 1:FD + 2],
                                in1=ut[:, FD:FD + 1], op=Alu.mult)       # Psq*L
        nc.vector.tensor_tensor(out=Z[0:64, 3:4], in0=vt[:, FD:FD + 1],
                                in1=ut[:, FD + 1:FD + 2], op=Alu.mult)   # Pbar*Lsq
        scrF = keep.tile([1, FD], f32, name="scrF")
        nc.vector.tensor_tensor(out=scrF[:], in0=last2[:, 0:FD],
                                in1=r1[:, 0:FD], op=Alu.mult)
        nc.vector.reduce_sum(out=Z[0:1, 4:5], in_=scrF[:], axis=Ax.X)  # Fe.F

        zred = psum.tile([1, 8], f32, name="zred", tag="zred")
        nc.tensor.matmul(out=zred[:], lhsT=ones128[:], rhs=Z[:],
                         start=True, stop=True)
        zs = keep.tile([1, 8], f32, name="zs")
        nc.vector.tensor_copy(out=zs[:], in_=zred[:])

        # scalars: M=last2[256], a=last2[257], e=r1[256], se=r1[257]
        Mv = last2[0:1, FD:FD + 1]
        av = last2[0:1, FD + 1:FD + 2]
        ev = r1[0:1, FD:FD + 1]
        sev = r1[0:1, FD + 1:FD + 2]
        s_center = zs[0:1, 0:1]
        uv = zs[0:1, 1:2]
        psql = zs[0:1, 2:3]
        pbarlsq = zs[0:1, 3:4]
        fef = zs[0:1, 4:5]

        fin = keep.tile([1, 16], f32, name="fin")
        t_ = lambda i: fin[0:1, i:i + 1]
        # f0 = se*M ; f1 = a*e ; f2 = f0+f1
        nc.vector.tensor_tensor(out=t_(8), in0=sev, in1=Mv, op=Alu.mult)
        nc.vector.tensor_tensor(out=t_(9), in0=av, in1=ev, op=Alu.mult)
        nc.vector.tensor_tensor(out=t_(10), in0=t_(8), in1=t_(9), op=Alu.add)
        # f3 = -2*fef + f2
        nc.vector.tensor_scalar(out=t_(11), in0=fef, scalar1=-2.0,
                                scalar2=t_(10), op0=Alu.mult, op1=Alu.add)
        # f4 = f3 - psql ; f5 = f4 - pbarlsq
        nc.vector.tensor_tensor(out=t_(12), in0=t_(11), in1=psql, op=Alu.subtract)
        nc.vector.tensor_tensor(out=t_(13), in0=t_(12), in1=pbarlsq, op=Alu.subtract)
        # SD = 2*uv + f5
        nc.vector.tensor_scalar(out=t_(14), in0=uv, scalar1=2.0,
                                scalar2=t_(13), op0=Alu.mult, op1=Alu.add)
        # md = M*(M-1) ; rmd = 1/md ; div = SD*rmd*(-1/D)
        nc.vector.tensor_scalar(out=t_(15), in0=Mv, scalar1=-1.0,
                                scalar2=Mv, op0=Alu.add, op1=Alu.mult)
        nc.vector.reciprocal(t_(15), t_(15))
        nc.vector.tensor_tensor(out=t_(1), in0=t_(14), in1=t_(15), op=Alu.mult)
        nc.vector.tensor_scalar_mul(out=t_(1), in0=t_(1), scalar1=-1.0 / D)
        # tight = (a - s_center)/M
        nc.vector.tensor_tensor(out=t_(7), in0=av, in1=s_center, op=Alu.subtract)
        nc.vector.reciprocal(t_(6), Mv)
        nc.vector.tensor_tensor(out=t_(2), in0=t_(7), in1=t_(6), op=Alu.mult)
        # total = 0.1*div + 0.1*tight
        nc.vector.tensor_tensor(out=t_(0), in0=t_(1), in1=t_(2), op=Alu.add)
        nc.vector.tensor_scalar_mul(out=t_(0), in0=t_(0), scalar1=0.1)
        # debug slots
        nc.vector.tensor_copy(out=t_(3), in_=Mv)
        nc.vector.tensor_copy(out=t_(4), in_=av)
        nc.vector.tensor_copy(out=t_(5), in_=sev)

        nc.sync.dma_start(out=outd[None, :], in_=fin[0:1, 0:8])

    nc.finalize()
    return nc


def _get_compiled():
    if "nc" not in _compiled:
        _compiled["nc"] = _build_bass()
    return _compiled["nc"]


def _make_in_maps(features, targets, mask):
    features = np.ascontiguousarray(np.asarray(features, dtype=np.float32))
    targets = np.ascontiguousarray(np.asarray(targets, dtype=np.float32))
    maskf = np.asarray(mask).astype(np.float32).reshape(B, 1)
    in_maps = []
    for i in range(NCORES):
        sl = slice(i * RB, (i + 1) * RB)
        in_maps.append({
            "features": features[sl],
            "targets": targets[sl],
            "maskf": np.ascontiguousarray(maskf[sl]),
        })
    return in_maps


def kernel(features, targets, mask):
    from concourse.bass_utils import run_bass_kernel_spmd

    nc = _get_compiled()
    in_maps = _make_in_maps(features, targets, mask)
    res = run_bass_kernel_spmd(nc, in_maps, list(range(NCORES)))
    out = res.results[0]["out"]
    total = np.float32(out[0])
    diversity = np.float32(out[1])
    tightness = np.float32(out[2])
    return total, diversity, tightness


# revision 17
# speedup vs baseline: 1.2598x; 1.2598x over previous
"""Trainium2 Bass kernel for CategoricalEntropyRegLoss.

Math: both loss terms factor so the [B,B] pairwise matrices are never built.

  feat_dists = sq_j + sq_k - 2 fn_j.fn_k            (rank FD+2)
  target_dists = (E_j - P_j.LQ_k) / D               (rank DC+1)
  S = sum_{jk} m_j m_k feat_dists * target_dists    (diag is exactly 0)
    = [ se*M + a*e - 2 Fe.F - Psq.L - Pbar.Lsq + 2 <U,V> ] / D
  tightness*M = a - sum_s ||seg_sum_s||^2 / max(cnt_s,1)

Everything needed is one matmul per core:
  out[1154, 258] = ext_seg^T @ ext_feat
  ext_seg  = [ onehot(code) | LQ | P | 1 | E ]      (B x 1154)
  ext_feat = [ m*fn | m | m*sq ]                    (B x 258)
followed by a single 8-core AllReduce of the [1154,258] partials and a
cheap redundant epilogue on every core.
"""

import numpy as np

B = 4096
FD = 256
C = 32
D = 2
NSEG = C ** D          # 1024
NCORES = 8
RB = B // NCORES       # 512 rows per core
KT = RB // 128         # 4 k-chunks of 128 rows
EF = FD + 2            # 258: [mfn | m | m*sq]
ES = NSEG + 2 * D * C + 2   # 1154: [onehot | LQ | P | ones | E]
PCOL = NSEG + D * C    # 1088: start of P block
LCOL = NSEG            # 1024: start of LQ block
ONES_COL = NSEG + 2 * D * C      # 1152
E_COL = ONES_COL + 1             # 1153
NMT = (ES + 127) // 128          # 10 m-tiles (last has 2 rows)

_compiled = {}


def _build_bass():
    from contextlib import ExitStack
    import concourse.bass as bass
    import concourse.bacc as bacc
    import concourse.tile as tile
    from concourse import mybir

    f32 = mybir.dt.float32
    Alu = mybir.AluOpType
    Act = mybir.ActivationFunctionType
    Ax = mybir.AxisListType

    nc = bacc.Bacc(num_devices=NCORES)

    feat = nc.dram_tensor("features", [RB, FD], f32, kind="ExternalInput")
    targ = nc.dram_tensor("targets", [RB, D * C], f32, kind="ExternalInput")
    maskf = nc.dram_tensor("maskf", [RB, 1], f32, kind="ExternalInput")
    outd = nc.dram_tensor("out", [8], f32, kind="ExternalOutput")

    with ExitStack() as ctx:
        tc = ctx.enter_context(tile.TileContext(nc))
        consts = ctx.enter_context(tc.tile_pool(name="consts", bufs=1))
        io = ctx.enter_context(tc.tile_pool(name="io", bufs=4))
        work = ctx.enter_context(tc.tile_pool(name="work", bufs=2))
        keep = ctx.enter_context(tc.tile_pool(name="keep", bufs=1))
        res_pool = ctx.enter_context(tc.tile_pool(name="res", bufs=3))
        psum = ctx.enter_context(tc.tile_pool(name="psum", bufs=2, space="PSUM"))
        dram = ctx.enter_context(tc.tile_pool(name="dram", bufs=1, space="DRAM"))

        # ---------------- constants ----------------
        iota1024 = consts.tile([128, NSEG], f32)
        nc.gpsimd.iota(iota1024[:], [[1, NSEG]], channel_multiplier=0,
                       allow_small_or_imprecise_dtypes=True)
        # biota[j] = 32 - j  (for first-argmax via reduce_max)
        biota = consts.tile([128, C], f32)
        nc.gpsimd.iota(biota[:], [[-1, C]], base=C, channel_multiplier=0,
                       allow_small_or_imprecise_dtypes=True)
        ones128 = consts.tile([128, 1], f32)
        nc.vector.memset(ones128[:], 1.0)

        inbounce = dram.tile([ES, EF], f32, name="inbounce")
        outbounce = dram.tile([ES, EF], f32, name="outbounce", addr_space="Shared")

        # ---- pass 0: load inputs, row sum-of-squares (DVE only) ----
        xs, ts_, mks = [], [], []
        sqpack = keep.tile([128, KT], f32, name="sqpack")
        for kc in range(KT):
            sl = slice(kc * 128, (kc + 1) * 128)
            x = io.tile([128, FD], f32, name=f"x_{kc}", tag="x")
            nc.sync.dma_start(out=x[:], in_=feat[sl, :])
            t = io.tile([128, D * C], f32, name=f"t_{kc}", tag="t")
            nc.sync.dma_start(out=t[:], in_=targ[sl, :])
            mk = io.tile([128, 1], f32, name=f"mk_{kc}", tag="mk")
            nc.sync.dma_start(out=mk[:], in_=maskf[sl, :])
            xs.append(x); ts_.append(t); mks.append(mk)
            scr = work.tile([128, FD], f32, name=f"scr_{kc}", tag="scr")
            nc.vector.tensor_tensor(out=scr[:], in0=x[:], in1=x[:], op=Alu.mult)
            nc.vector.reduce_sum(out=sqpack[:, kc:kc + 1], in_=scr[:], axis=Ax.X)

        # one Sqrt for all chunks -> one ACT table load
        normpack = keep.tile([128, KT], f32, name="normpack")
        nc.scalar.sqrt(normpack[:], sqpack[:])
        nc.vector.tensor_scalar_max(out=normpack[:], in0=normpack[:],
                                    scalar1=1e-12)
        invpack = keep.tile([128, KT], f32, name="invpack")
        nc.vector.reciprocal(invpack[:], normpack[:])

        es_tiles = []
        ef_tiles = []
        for kc in range(KT):
            x, t, mk = xs[kc], ts_[kc], mks[kc]
            inv = invpack[:, kc:kc + 1]

            ef_t = keep.tile([128, EF], f32, name=f"ef_{kc}")
            es_t = keep.tile([128, ES], f32, name=f"es_{kc}")

            # ---- ext_feat = [m*fn | m | m*sq],  sq = sq0*inv^2 ----
            fn = work.tile([128, FD], f32, name=f"fn_{kc}", tag="fn")
            nc.vector.tensor_scalar_mul(out=fn[:], in0=x[:], scalar1=inv)
            sq = work.tile([128, 1], f32, name=f"sq_{kc}", tag="sq")
            nc.vector.tensor_scalar(out=sq[:], in0=sqpack[:, kc:kc + 1],
                                    scalar1=inv, scalar2=inv,
                                    op0=Alu.mult, op1=Alu.mult)
            nc.vector.tensor_scalar_mul(out=ef_t[:, 0:FD], in0=fn[:], scalar1=mk[:])
            nc.vector.tensor_copy(out=ef_t[:, FD:FD + 1], in_=mk[:])
            nc.vector.tensor_tensor(out=ef_t[:, FD + 1:FD + 2], in0=mk[:],
                                    in1=sq[:], op=Alu.mult)

            # ---- target probs / logs / entropy ----
            t1 = work.tile([128, D * C], f32, name=f"t1_{kc}", tag="t1")
            nc.vector.tensor_scalar_add(out=t1[:], in0=t[:], scalar1=1e-10)
            ssum = work.tile([128, D], f32, name=f"ssum_{kc}", tag="ssum")
            nc.vector.reduce_sum(out=ssum[:],
                                 in_=t1[:].rearrange("p (d c) -> p d c", c=C),
                                 axis=Ax.X)
            invs = work.tile([128, D], f32, name=f"invs_{kc}", tag="invs")
            nc.vector.reciprocal(invs[:], ssum[:])
            for d_ in range(D):
                nc.vector.tensor_scalar_mul(
                    out=es_t[:, PCOL + C * d_:PCOL + C * (d_ + 1)],
                    in0=t1[:, C * d_:C * (d_ + 1)],
                    scalar1=invs[:, d_:d_ + 1])
            nc.scalar.activation(out=es_t[:, LCOL:LCOL + D * C],
                                 in_=es_t[:, PCOL:PCOL + D * C], func=Act.Ln)
            scr64 = work.tile([128, D * C], f32, name=f"scr64_{kc}", tag="scr64")
            nc.vector.tensor_tensor(
                out=scr64[:], in0=es_t[:, PCOL:PCOL + D * C],
                in1=es_t[:, LCOL:LCOL + D * C], op=Alu.mult)
            nc.vector.reduce_sum(out=es_t[:, E_COL:E_COL + 1], in_=scr64[:],
                                 axis=Ax.X)
            nc.vector.memset(es_t[:, ONES_COL:ONES_COL + 1], 1.0)

            # ---- first-argmax per dim, then code = cls0 + 32*cls1 ----
            cls = work.tile([128, D], f32, name=f"cls_{kc}", tag="cls")
            for d_ in range(D):
                pch = es_t[:, PCOL + C * d_:PCOL + C * (d_ + 1)]
                mx = work.tile([128, 1], f32, name=f"mx_{kc}_{d_}", tag="mx")
                nc.vector.reduce_max(out=mx[:], in_=pch, axis=Ax.X)
                cand = work.tile([128, C], f32, name=f"cand_{kc}_{d_}", tag="cand")
                # (p == max) * (32 - idx); reduce_max -> 32 - first_argmax
                nc.vector.scalar_tensor_tensor(
                    out=cand[:], in0=pch, scalar=mx[:], in1=biota[:],
                    op0=Alu.is_equal, op1=Alu.mult)
                mq = work.tile([128, 1], f32, name=f"mq_{kc}_{d_}", tag="mq")
                nc.vector.reduce_max(out=mq[:], in_=cand[:], axis=Ax.X)
                nc.vector.tensor_scalar(
                    out=cls[:, d_:d_ + 1], in0=mq[:], scalar1=-1.0,
                    scalar2=float(C), op0=Alu.mult, op1=Alu.add)
            code = work.tile([128, 1], f32, name=f"code_{kc}", tag="code")
            nc.vector.tensor_scalar(
                out=code[:], in0=cls[:, 1:2], scalar1=float(C),
                scalar2=cls[:, 0:1], op0=Alu.mult, op1=Alu.add)
            # ---- one-hot ----
            nc.vector.tensor_scalar(
                out=es_t[:, 0:NSEG], in0=iota1024[:], scalar1=code[:],
                scalar2=None, op0=Alu.is_equal)

            es_tiles.append(es_t)
            ef_tiles.append(ef_t)

        # ---------------- the one big matmul ----------------
        for mt in range(NMT):
            mlo = mt * 128
            msz = min(128, ES - mlo)
            ps = psum.tile([msz, EF], f32, name=f"ps_{mt}", tag="ps")
            for kc in range(KT):
                nc.tensor.matmul(
                    out=ps[:], lhsT=es_tiles[kc][:, mlo:mlo + msz],
                    rhs=ef_tiles[kc][:], start=(kc == 0), stop=(kc == KT - 1))
            rt = res_pool.tile([msz, EF], f32, name=f"rt_{mt}", tag="rt")
            nc.vector.tensor_copy(out=rt[:], in_=ps[:])
            nc.sync.dma_start(out=inbounce[mlo:mlo + msz, :], in_=rt[:])

        # ---------------- single AllReduce ----------------
        nc.gpsimd.collective_compute(
            "AllReduce", mybir.AluOpType.add,
            replica_groups=[list(range(NCORES))],
            ins=[inbounce.opt()], outs=[outbounce.opt()])

        # ---------------- epilogue (redundant on every core) ----------------
        # two big strided loads: rows 0:512 and 512:1152 as [128, a, EF]
        big0 = keep.tile([128, 4, EF], f32, name="big0")
        nc.sync.dma_start(
            out=big0[:],
            in_=outbounce[0:512, :].rearrange("(a p) f -> p a f", p=128))
        big1 = keep.tile([128, 5, EF], f32, name="big1")
        nc.sync.dma_start(
            out=big1[:],
            in_=outbounce[512:1152, :].rearrange("(a p) f -> p a f", p=128))
        last2 = keep.tile([1, EF], f32, name="last2")
        nc.sync.dma_start(out=last2[:], in_=outbounce[ONES_COL:ONES_COL + 1, :])
        r1 = keep.tile([1, EF], f32, name="r1")
        nc.sync.dma_start(out=r1[:], in_=outbounce[E_COL:E_COL + 1, :])

        Z = keep.tile([128, 8], f32, name="Z")
        nc.vector.memset(Z[:], 0.0)
        acc = keep.tile([128, 1], f32, name="acc")
        nc.vector.memset(acc[:], 0.0)
        nrmp = keep.tile([128, 8], f32, name="nrmp")
        cdp = keep.tile([128, 8], f32, name="cdp")
        for st in range(NSEG // 128):
            sgt = (big0[:, st, :] if st < 4 else big1[:, st - 4, :])
            scrP = work.tile([128, FD], f32, name=f"scrP_{st}", tag="scrP")
            nc.scalar.activation(out=scrP[:], in_=sgt[:, 0:FD],
                                 func=Act.Square, accum_out=nrmp[:, st:st + 1])
            nc.vector.tensor_scalar_max(out=cdp[:, st:st + 1],
                                        in0=sgt[:, FD:FD + 1], scalar1=1.0)
        rcdp = keep.tile([128, 8], f32, name="rcdp")
        nc.vector.reciprocal(rcdp[:], cdp[:])
        termp = keep.tile([128, 8], f32, name="termp")
        nc.vector.tensor_tensor(out=termp[:], in0=nrmp[:], in1=rcdp[:],
                                op=Alu.mult)
        nc.vector.reduce_sum(out=Z[:, 0:1], in_=termp[:], axis=Ax.X)

        # rows 1024:1152 live at big1[:, 4, :]: partitions 0:64 = U^T rows
        vt = keep.tile([64, EF], f32, name="vt")
        nc.sync.dma_start(out=vt[:], in_=outbounce[PCOL:PCOL + D * C, :])

        scrU = keep.tile([64, FD], f32, name="scrU")
        nc.vector.tensor_tensor(out=scrU[:], in0=big1[0:64, 4, 0:FD],
                                in1=vt[:, 0:FD], op=Alu.mult)
        nc.vector.reduce_sum(out=Z[0:64, 1:2], in_=scrU[:], axis=Ax.X)
        nc.vector.tensor_tensor(out=Z[0:64, 2:3], in0=vt[:, FD + 1:FD + 2],
                                in1=big1[0:64, 4, FD:FD + 1], op=Alu.mult)     # Psq*L
        nc.vector.tensor_tensor(out=Z[0:64, 3:4], in0=vt[:, FD:FD + 1],
                                in1=big1[0:64, 4, FD + 1:FD + 2], op=Alu.mult)  # Pbar*Lsq
        scrF = keep.tile([1, FD], f32, name="scrF")
        nc.vector.tensor_tensor(out=scrF[:], in0=last2[:, 0:FD],
                                in1=r1[:, 0:FD], op=Alu.mult)
        nc.vector.reduce_sum(out=Z[0:1, 4:5], in_=scrF[:], axis=Ax.X)  # Fe.F

        zred = psum.tile([1, 8], f32, name="zred", tag="zred")
        nc.tensor.matmul(out=zred[:], lhsT=ones128[:], rhs=Z[:],
                         start=True, stop=True)
        zs = keep.tile([1, 8], f32, name="zs")
        nc.vector.tensor_copy(out=zs[:], in_=zred[:])

        # scalars: M=last2[256], a=last2[257], e=r1[256], se=r1[257]
        Mv = last2[0:1, FD:FD + 1]
        av = last2[0:1, FD + 1:FD + 2]
        ev = r1[0:1, FD:FD + 1]
        sev = r1[0:1, FD + 1:FD + 2]
        s_center = zs[0:1, 0:1]
        uv = zs[0:1, 1:2]
        psql = zs[0:1, 2:3]
        pbarlsq = zs[0:1, 3:4]
        fef = zs[0:1, 4:5]

        fin = keep.tile([1, 16], f32, name="fin")
        t_ = lambda i: fin[0:1, i:i + 1]
        # f0 = se*M ; f1 = a*e ; f2 = f0+f1
        nc.vector.tensor_tensor(out=t_(8), in0=sev, in1=Mv, op=Alu.mult)
        nc.vector.tensor_tensor(out=t_(9), in0=av, in1=ev, op=Alu.mult)
        nc.vector.tensor_tensor(out=t_(10), in0=t_(8), in1=t_(9), op=Alu.add)
        # f3 = -2*fef + f2
        nc.vector.tensor_scalar(out=t_(11), in0=fef, scalar1=-2.0,
                                scalar2=t_(10), op0=Alu.mult, op1=Alu.add)
        # f4 = f3 - psql ; f5 = f4 - pbarlsq
        nc.vector.tensor_tensor(out=t_(12), in0=t_(11), in1=psql, op=Alu.subtract)
        nc.vector.tensor_tensor(out=t_(13), in0=t_(12), in1=pbarlsq, op=Alu.subtract)
        # SD = 2*uv + f5
        nc.vector.tensor_scalar(out=t_(14), in0=uv, scalar1=2.0,
                                scalar2=t_(13), op0=Alu.mult, op1=Alu.add)
        # md = M*(M-1) ; rmd = 1/md ; div = SD*rmd*(-1/D)
        nc.vector.tensor_scalar(out=t_(15), in0=Mv, scalar1=-1.0,
                                scalar2=Mv, op0=Alu.add, op1=Alu.mult)
        nc.vector.reciprocal(t_(15), t_(15))
        nc.vector.tensor_tensor(out=t_(1), in0=t_(14), in1=t_(15), op=Alu.mult)
        nc.vector.tensor_scalar_mul(out=t_(1), in0=t_(1), scalar1=-1.0 / D)
        # tight = (a - s_center)/M
        nc.vector.tensor_tensor(out=t_(7), in0=av, in1=s_center, op=Alu.subtract)
        nc.vector.reciprocal(t_(6), Mv)
        nc.vector.tensor_tensor(out=t_(2), in0=t_(7), in1=t_(6), op=Alu.mult)
        # total = 0.1*div + 0.1*tight
        nc.vector.tensor_tensor(out=t_(0), in0=t_(1), in1=t_(2), op=Alu.add)
        nc.vector.tensor_scalar_mul(out=t_(0), in0=t_(0), scalar1=0.1)
        # debug slots
        nc.vector.tensor_copy(out=t_(3), in_=Mv)
        nc.vector.tensor_copy(out=t_(4), in_=av)
        nc.vector.tensor_copy(out=t_(5), in_=sev)

        nc.sync.dma_start(out=outd[None, :], in_=fin[0:1, 0:8])

    nc.finalize()
    return nc


def _get_compiled():
    if "nc" not in _compiled:
        _compiled["nc"] = _build_bass()
    return _compiled["nc"]


def _make_in_maps(features, targets, mask):
    features = np.ascontiguousarray(np.asarray(features, dtype=np.float32))
    targets = np.ascontiguousarray(np.asarray(targets, dtype=np.float32))
    maskf = np.asarray(mask).astype(np.float32).reshape(B, 1)
    in_maps = []
    for i in range(NCORES):
        sl = slice(i * RB, (i + 1) * RB)
        in_maps.append({
            "features": features[sl],
            "targets": targets[sl],
            "maskf": np.ascontiguousarray(maskf[sl]),
        })
    return in_maps


def kernel(features, targets, mask):
    from concourse.bass_utils import run_bass_kernel_spmd

    nc = _get_compiled()
    in_maps = _make_in_maps(features, targets, mask)
    res = run_bass_kernel_spmd(nc, in_maps, list(range(NCORES)))
    out = res.results[0]["out"]
    total = np.float32(out[0])
    diversity = np.float32(out[1])
    tightness = np.float32(out[2])
    return total, diversity, tightness


# revision 19
# speedup vs baseline: 1.2955x; 1.0284x over previous
"""Trainium2 Bass kernel for CategoricalEntropyRegLoss.

Math: both loss terms factor so the [B,B] pairwise matrices are never built.

  feat_dists = sq_j + sq_k - 2 fn_j.fn_k            (rank FD+2)
  target_dists = (E_j - P_j.LQ_k) / D               (rank DC+1)
  S = sum_{jk} m_j m_k feat_dists * target_dists    (diag is exactly 0)
    = [ se*M + a*e - 2 Fe.F - Psq.L - Pbar.Lsq + 2 <U,V> ] / D
  tightness*M = a - sum_s ||seg_sum_s||^2 / max(cnt_s,1)

Everything needed is one matmul per core:
  out[1154, 258] = ext_seg^T @ ext_feat
  ext_seg  = [ onehot(code) | LQ | P | 1 | E ]      (B x 1154)
  ext_feat = [ m*fn | m | m*sq ]                    (B x 258)
followed by a single 8-core AllReduce of the [1154,258] partials and a
cheap redundant epilogue on every core.
"""

import numpy as np

B = 4096
FD = 256
C = 32
D = 2
NSEG = C ** D          # 1024
NCORES = 8
RB = B // NCORES       # 512 rows per core
KT = RB // 128         # 4 k-chunks of 128 rows
EF = FD + 2            # 258: [mfn | m | m*sq]
ES = NSEG + 2 * D * C + 2   # 1154: [onehot | LQ | P | ones | E]
PCOL = NSEG + D * C    # 1088: start of P block
LCOL = NSEG            # 1024: start of LQ block
ONES_COL = NSEG + 2 * D * C      # 1152
E_COL = ONES_COL + 1             # 1153
NMT = (ES + 127) // 128          # 10 m-tiles (last has 2 rows)

_compiled = {}


def _build_bass():
    from contextlib import ExitStack
    import concourse.bass as bass
    import concourse.bacc as bacc
    import concourse.tile as tile
    from concourse import mybir

    f32 = mybir.dt.float32
    Alu = mybir.AluOpType
    Act = mybir.ActivationFunctionType
    Ax = mybir.AxisListType

    nc = bacc.Bacc(num_devices=NCORES)

    feat = nc.dram_tensor("features", [RB, FD], f32, kind="ExternalInput")
    targ = nc.dram_tensor("targets", [RB, D * C], f32, kind="ExternalInput")
    maskf = nc.dram_tensor("maskf", [RB, 1], f32, kind="ExternalInput")
    outd = nc.dram_tensor("out", [8], f32, kind="ExternalOutput")

    with ExitStack() as ctx:
        tc = ctx.enter_context(tile.TileContext(nc))
        consts = ctx.enter_context(tc.tile_pool(name="consts", bufs=1))
        work = ctx.enter_context(tc.tile_pool(name="work", bufs=1))
        keep = ctx.enter_context(tc.tile_pool(name="keep", bufs=1))
        res_pool = ctx.enter_context(tc.tile_pool(name="res", bufs=1))
        psum = ctx.enter_context(tc.tile_pool(name="psum", bufs=1, space="PSUM"))
        dram = ctx.enter_context(tc.tile_pool(name="dram", bufs=1, space="DRAM"))

        # ---------------- constants ----------------
        iota1024 = consts.tile([128, NSEG], f32)
        nc.gpsimd.iota(iota1024[:], [[1, NSEG]], channel_multiplier=0,
                       allow_small_or_imprecise_dtypes=True)
        # biota[j] = 32 - j  (for first-argmax via reduce_max)
        biota = consts.tile([128, C], f32)
        nc.gpsimd.iota(biota[:], [[-1, C]], base=C, channel_multiplier=0,
                       allow_small_or_imprecise_dtypes=True)
        ones128 = consts.tile([128, 1], f32)
        nc.vector.memset(ones128[:], 1.0)

        inbounce = dram.tile([ES, EF], f32, name="inbounce")
        outbounce = dram.tile([ES, EF], f32, name="outbounce", addr_space="Shared")

        # ---- batched input loads: 3 DMAs total (t first, x biggest last) ----
        tbig = keep.tile([128, KT, D * C], f32, name="tbig")
        nc.sync.dma_start(
            out=tbig[:], in_=targ[:, :].rearrange("(a p) f -> p a f", p=128))
        mkbig = keep.tile([128, KT, 1], f32, name="mkbig")
        nc.sync.dma_start(
            out=mkbig[:], in_=maskf[:, :].rearrange("(a p) f -> p a f", p=128))
        xbig = keep.tile([128, KT, FD], f32, name="xbig")
        nc.sync.dma_start(
            out=xbig[:], in_=feat[:, :].rearrange("(a p) f -> p a f", p=128))

        es_tiles = [keep.tile([128, ES], f32, name=f"es_{kc}")
                    for kc in range(KT)]
        ef_tiles = [keep.tile([128, EF], f32, name=f"ef_{kc}")
                    for kc in range(KT)]

        # ---- ACT phase 1: row sum-of-squares (Square table loads once) ----
        sqpack = keep.tile([128, KT], f32, name="sqpack")
        scrsq = keep.tile([128, FD], f32, name="scrsq")
        for kc in range(KT):
            nc.scalar.activation(out=scrsq[:], in_=xbig[:, kc, :],
                                 func=Act.Square,
                                 accum_out=sqpack[:, kc:kc + 1])
        # ---- ACT phase 2: one Sqrt for all chunks ----
        normpack = keep.tile([128, KT], f32, name="normpack")
        nc.scalar.sqrt(normpack[:], sqpack[:])
        nc.vector.tensor_scalar_max(out=normpack[:], in0=normpack[:],
                                    scalar1=1e-12)
        invpack = keep.tile([128, KT], f32, name="invpack")
        nc.vector.reciprocal(invpack[:], normpack[:])
        # minv = m * inv  (fold mask into the normalization scale)
        minvpack = keep.tile([128, KT], f32, name="minvpack")
        nc.vector.tensor_tensor(out=minvpack[:], in0=invpack[:],
                                in1=mkbig[:, :, 0], op=Alu.mult)

        # ---- targets chains (DVE) + Ln (ACT phase 3) ----
        for kc in range(KT):
            es_t = es_tiles[kc]
            t1 = work.tile([128, D * C], f32, name=f"t1_{kc}", tag=f"t1_{kc}")
            nc.vector.tensor_scalar_add(out=t1[:], in0=tbig[:, kc, :],
                                        scalar1=1e-10)
            ssum = work.tile([128, D], f32, name=f"ssum_{kc}", tag=f"ss_{kc}")
            nc.vector.reduce_sum(out=ssum[:],
                                 in_=t1[:].rearrange("p (d c) -> p d c", c=C),
                                 axis=Ax.X)
            invs = work.tile([128, D], f32, name=f"invs_{kc}", tag=f"iv_{kc}")
            nc.vector.reciprocal(invs[:], ssum[:])
            for d_ in range(D):
                nc.vector.tensor_scalar_mul(
                    out=es_t[:, PCOL + C * d_:PCOL + C * (d_ + 1)],
                    in0=t1[:, C * d_:C * (d_ + 1)],
                    scalar1=invs[:, d_:d_ + 1])
            nc.scalar.activation(out=es_t[:, LCOL:LCOL + D * C],
                                 in_=es_t[:, PCOL:PCOL + D * C], func=Act.Ln)
            scr64 = work.tile([128, D * C], f32, name=f"scr64_{kc}",
                              tag=f"s64_{kc}")
            nc.vector.tensor_tensor(
                out=scr64[:], in0=es_t[:, PCOL:PCOL + D * C],
                in1=es_t[:, LCOL:LCOL + D * C], op=Alu.mult)
            nc.vector.reduce_sum(out=es_t[:, E_COL:E_COL + 1], in_=scr64[:],
                                 axis=Ax.X)
            nc.vector.memset(es_t[:, ONES_COL:ONES_COL + 1], 1.0)

            # ---- first-argmax per dim, then code = cls0 + 32*cls1 ----
            cls = work.tile([128, D], f32, name=f"cls_{kc}", tag=f"cl_{kc}")
            for d_ in range(D):
                pch = es_t[:, PCOL + C * d_:PCOL + C * (d_ + 1)]
                mx = work.tile([128, 1], f32, name=f"mx_{kc}_{d_}",
                               tag=f"mx_{kc}_{d_}")
                nc.vector.reduce_max(out=mx[:], in_=pch, axis=Ax.X)
                cand = work.tile([128, C], f32, name=f"cand_{kc}_{d_}",
                                 tag=f"cd_{kc}_{d_}")
                # (p == max) * (32 - idx); reduce_max -> 32 - first_argmax
                nc.vector.scalar_tensor_tensor(
                    out=cand[:], in0=pch, scalar=mx[:], in1=biota[:],
                    op0=Alu.is_equal, op1=Alu.mult)
                mq = work.tile([128, 1], f32, name=f"mq_{kc}_{d_}",
                               tag=f"mq_{kc}_{d_}")
                nc.vector.reduce_max(out=mq[:], in_=cand[:], axis=Ax.X)
                nc.vector.tensor_scalar(
                    out=cls[:, d_:d_ + 1], in0=mq[:], scalar1=-1.0,
                    scalar2=float(C), op0=Alu.mult, op1=Alu.add)
            code = work.tile([128, 1], f32, name=f"code_{kc}", tag=f"co_{kc}")
            nc.vector.tensor_scalar(
                out=code[:], in0=cls[:, 1:2], scalar1=float(C),
                scalar2=cls[:, 0:1], op0=Alu.mult, op1=Alu.add)
            # ---- one-hot ----
            nc.vector.tensor_scalar(
                out=es_t[:, 0:NSEG], in0=iota1024[:], scalar1=code[:],
                scalar2=None, op0=Alu.is_equal)

        # ---- ext_feat = [x*(m*inv) | m | sq0*inv*minv] (ACT phase 4) ----
        for kc in range(KT):
            ef_t = ef_tiles[kc]
            nc.scalar.activation(out=ef_t[:, 0:FD], in_=xbig[:, kc, :],
                                 func=Act.Copy,
                                 scale=minvpack[:, kc:kc + 1])
            nc.vector.tensor_copy(out=ef_t[:, FD:FD + 1], in_=mkbig[:, kc, :])
            nc.vector.tensor_scalar(out=ef_t[:, FD + 1:FD + 2],
                                    in0=sqpack[:, kc:kc + 1],
                                    scalar1=invpack[:, kc:kc + 1],
                                    scalar2=minvpack[:, kc:kc + 1],
                                    op0=Alu.mult, op1=Alu.mult)

        # ---------------- the one big matmul ----------------
        for mt in range(NMT):
            mlo = mt * 128
            msz = min(128, ES - mlo)
            ps = psum.tile([msz, EF], f32, name=f"ps_{mt}", tag=f"ps_{mt % 6}")
            for kc in range(KT):
                nc.tensor.matmul(
                    out=ps[:], lhsT=es_tiles[kc][:, mlo:mlo + msz],
                    rhs=ef_tiles[kc][:], start=(kc == 0), stop=(kc == KT - 1))
            rt = res_pool.tile([msz, EF], f32, name=f"rt_{mt}", tag=f"rt_{mt}")
            nc.vector.tensor_copy(out=rt[:], in_=ps[:])
            nc.sync.dma_start(out=inbounce[mlo:mlo + msz, :], in_=rt[:])

        # ---------------- single AllReduce ----------------
        nc.gpsimd.collective_compute(
            "AllReduce", mybir.AluOpType.add,
            replica_groups=[list(range(NCORES))],
            ins=[inbounce.opt()], outs=[outbounce.opt()])

        # ---------------- epilogue (redundant on every core) ----------------
        # two big strided loads: rows 0:512 and 512:1152 as [128, a, EF]
        big0 = keep.tile([128, 4, EF], f32, name="big0")
        nc.sync.dma_start(
            out=big0[:],
            in_=outbounce[0:512, :].rearrange("(a p) f -> p a f", p=128))
        big1 = keep.tile([128, 5, EF], f32, name="big1")
        nc.sync.dma_start(
            out=big1[:],
            in_=outbounce[512:1152, :].rearrange("(a p) f -> p a f", p=128))
        last2 = keep.tile([1, EF], f32, name="last2")
        nc.sync.dma_start(out=last2[:], in_=outbounce[ONES_COL:ONES_COL + 1, :])
        r1 = keep.tile([1, EF], f32, name="r1")
        nc.sync.dma_start(out=r1[:], in_=outbounce[E_COL:E_COL + 1, :])

        Z = keep.tile([128, 8], f32, name="Z")
        nc.vector.memset(Z[:], 0.0)
        acc = keep.tile([128, 1], f32, name="acc")
        nc.vector.memset(acc[:], 0.0)
        nrmp = keep.tile([128, 8], f32, name="nrmp")
        cdp = keep.tile([128, 8], f32, name="cdp")
        for st in range(NSEG // 128):
            sgt = (big0[:, st, :] if st < 4 else big1[:, st - 4, :])
            scrP = work.tile([128, FD], f32, name=f"scrP_{st}", tag="scrP")
            nc.scalar.activation(out=scrP[:], in_=sgt[:, 0:FD],
                                 func=Act.Square, accum_out=nrmp[:, st:st + 1])
            nc.vector.tensor_scalar_max(out=cdp[:, st:st + 1],
                                        in0=sgt[:, FD:FD + 1], scalar1=1.0)
        rcdp = keep.tile([128, 8], f32, name="rcdp")
        nc.vector.reciprocal(rcdp[:], cdp[:])
        termp = keep.tile([128, 8], f32, name="termp")
        nc.vector.tensor_tensor(out=termp[:], in0=nrmp[:], in1=rcdp[:],
                                op=Alu.mult)
        nc.vector.reduce_sum(out=Z[:, 0:1], in_=termp[:], axis=Ax.X)

        # rows 1024:1152 live at big1[:, 4, :]: partitions 0:64 = U^T rows
        vt = keep.tile([64, EF], f32, name="vt")
        nc.sync.dma_start(out=vt[:], in_=outbounce[PCOL:PCOL + D * C, :])

        scrU = keep.tile([64, FD], f32, name="scrU")
        nc.vector.tensor_tensor(out=scrU[:], in0=big1[0:64, 4, 0:FD],
                                in1=vt[:, 0:FD], op=Alu.mult)
        nc.vector.reduce_sum(out=Z[0:64, 1:2], in_=scrU[:], axis=Ax.X)
        nc.vector.tensor_tensor(out=Z[0:64, 2:3], in0=vt[:, FD + 1:FD + 2],
                                in1=big1[0:64, 4, FD:FD + 1], op=Alu.mult)     # Psq*L
        nc.vector.tensor_tensor(out=Z[0:64, 3:4], in0=vt[:, FD:FD + 1],
                                in1=big1[0:64, 4, FD + 1:FD + 2], op=Alu.mult)  # Pbar*Lsq
        scrF = keep.tile([1, FD], f32, name="scrF")
        nc.vector.tensor_tensor(out=scrF[:], in0=last2[:, 0:FD],
                                in1=r1[:, 0:FD], op=Alu.mult)
        nc.vector.reduce_sum(out=Z[0:1, 4:5], in_=scrF[:], axis=Ax.X)  # Fe.F

        zred = psum.tile([1, 8], f32, name="zred", tag="zred")
        nc.tensor.matmul(out=zred[:], lhsT=ones128[:], rhs=Z[:],
                         start=True, stop=True)
        zs = keep.tile([1, 8], f32, name="zs")
        nc.vector.tensor_copy(out=zs[:], in_=zred[:])

        # scalars: M=last2[256], a=last2[257], e=r1[256], se=r1[257]
        Mv = last2[0:1, FD:FD + 1]
        av = last2[0:1, FD + 1:FD + 2]
        ev = r1[0:1, FD:FD + 1]
        sev = r1[0:1, FD + 1:FD + 2]
        s_center = zs[0:1, 0:1]
        uv = zs[0:1, 1:2]
        psql = zs[0:1, 2:3]
        pbarlsq = zs[0:1, 3:4]
        fef = zs[0:1, 4:5]

        fin = keep.tile([1, 16], f32, name="fin")
        t_ = lambda i: fin[0:1, i:i + 1]
        # f0 = se*M ; f1 = a*e ; f2 = f0+f1
        nc.vector.tensor_tensor(out=t_(8), in0=sev, in1=Mv, op=Alu.mult)
        nc.vector.tensor_tensor(out=t_(9), in0=av, in1=ev, op=Alu.mult)
        nc.vector.tensor_tensor(out=t_(10), in0=t_(8), in1=t_(9), op=Alu.add)
        # f3 = -2*fef + f2
        nc.vector.tensor_scalar(out=t_(11), in0=fef, scalar1=-2.0,
                                scalar2=t_(10), op0=Alu.mult, op1=Alu.add)
        # f4 = f3 - psql ; f5 = f4 - pbarlsq
        nc.vector.tensor_tensor(out=t_(12), in0=t_(11), in1=psql, op=Alu.subtract)
        nc.vector.tensor_tensor(out=t_(13), in0=t_(12), in1=pbarlsq, op=Alu.subtract)
        # SD = 2*uv + f5
        nc.vector.tensor_scalar(out=t_(14), in0=uv, scalar1=2.0,
                                scalar2=t_(13), op0=Alu.mult, op1=Alu.add)
        # md = M*(M-1) ; rmd = 1/md ; div = SD*rmd*(-1/D)
        nc.vector.tensor_scalar(out=t_(15), in0=Mv, scalar1=-1.0,
                                scalar2=Mv, op0=Alu.add, op1=Alu.mult)
        nc.vector.reciprocal(t_(15), t_(15))
        nc.vector.tensor_tensor(out=t_(1), in0=t_(14), in1=t_(15), op=Alu.mult)
        nc.vector.tensor_scalar_mul(out=t_(1), in0=t_(1), scalar1=-1.0 / D)
        # tight = (a - s_center)/M
        nc.vector.tensor_tensor(out=t_(7), in0=av, in1=s_center, op=Alu.subtract)
        nc.vector.reciprocal(t_(6), Mv)
        nc.vector.tensor_tensor(out=t_(2), in0=t_(7), in1=t_(6), op=Alu.mult)
        # total = 0.1*div + 0.1*tight
        nc.vector.tensor_tensor(out=t_(0), in0=t_(1), in1=t_(2), op=Alu.add)
        nc.vector.tensor_scalar_mul(out=t_(0), in0=t_(0), scalar1=0.1)
        # debug slots
        nc.vector.tensor_copy(out=t_(3), in_=Mv)
        nc.vector.tensor_copy(out=t_(4), in_=av)
        nc.vector.tensor_copy(out=t_(5), in_=sev)

        nc.sync.dma_start(out=outd[None, :], in_=fin[0:1, 0:8])

    nc.finalize()
    return nc


def _get_compiled():
    if "nc" not in _compiled:
        _compiled["nc"] = _build_bass()
    return _compiled["nc"]


def _make_in_maps(features, targets, mask):
    features = np.ascontiguousarray(np.asarray(features, dtype=np.float32))
    targets = np.ascontiguousarray(np.asarray(targets, dtype=np.float32))
    maskf = np.asarray(mask).astype(np.float32).reshape(B, 1)
    in_maps = []
    for i in range(NCORES):
        sl = slice(i * RB, (i + 1) * RB)
        in_maps.append({
            "features": features[sl],
            "targets": targets[sl],
            "maskf": np.ascontiguousarray(maskf[sl]),
        })
    return in_maps


def kernel(features, targets, mask):
    from concourse.bass_utils import run_bass_kernel_spmd

    nc = _get_compiled()
    in_maps = _make_in_maps(features, targets, mask)
    res = run_bass_kernel_spmd(nc, in_maps, list(range(NCORES)))
    out = res.results[0]["out"]
    total = np.float32(out[0])
    diversity = np.float32(out[1])
    tightness = np.float32(out[2])
    return total, diversity, tightness


# revision 26
# speedup vs baseline: 1.3274x; 1.0246x over previous
"""Trainium2 Bass kernel for CategoricalEntropyRegLoss.

Math: both loss terms factor so the [B,B] pairwise matrices are never built.

  feat_dists = sq_j + sq_k - 2 fn_j.fn_k            (rank FD+2)
  target_dists = (E_j - P_j.LQ_k) / D               (rank DC+1)
  S = sum_{jk} m_j m_k feat_dists * target_dists    (diag is exactly 0)
    = [ se*M + a*e - 2 Fe.F - Psq.L - Pbar.Lsq + 2 <U,V> ] / D
  tightness*M = a - sum_s ||seg_sum_s||^2 / max(cnt_s,1)

Everything needed is one matmul per core:
  out[1154, 258] = ext_seg^T @ ext_feat
  ext_seg  = [ onehot(code) | LQ | P | 1 | E ]      (B x 1154)
  ext_feat = [ m*fn | m | m*sq ]                    (B x 258)
followed by a single 8-core AllReduce of the [1154,258] partials and a
cheap redundant epilogue on every core.
"""

import numpy as np

B = 4096
FD = 256
C = 32
D = 2
NSEG = C ** D          # 1024
NCORES = 8
RB = B // NCORES       # 512 rows per core
KT = RB // 128         # 4 k-chunks of 128 rows
EF = FD + 2            # 258: [mfn | m | m*sq]
ES = NSEG + 2 * D * C + 2   # 1154: [onehot | LQ | P | ones | E]
PCOL = NSEG + D * C    # 1088: start of P block
LCOL = NSEG            # 1024: start of LQ block
ONES_COL = NSEG + 2 * D * C      # 1152
E_COL = ONES_COL + 1             # 1153
NMT = (ES + 127) // 128          # 10 m-tiles (last has 2 rows)

_compiled = {}


def _build_bass():
    from contextlib import ExitStack
    import concourse.bass as bass
    import concourse.bacc as bacc
    import concourse.tile as tile
    from concourse import mybir

    from concourse.tile import add_dep_helper

    f32 = mybir.dt.float32
    bf16 = mybir.dt.bfloat16
    Alu = mybir.AluOpType
    Act = mybir.ActivationFunctionType
    Ax = mybir.AxisListType

    nc = bacc.Bacc(num_devices=NCORES)

    feat = nc.dram_tensor("features", [RB, FD], f32, kind="ExternalInput")
    targ = nc.dram_tensor("targets", [RB, D * C], f32, kind="ExternalInput")
    maskf = nc.dram_tensor("maskf", [RB, 1], f32, kind="ExternalInput")
    outd = nc.dram_tensor("out", [8], f32, kind="ExternalOutput")

    with ExitStack() as ctx:
        tc = ctx.enter_context(tile.TileContext(nc))
        consts = ctx.enter_context(tc.tile_pool(name="consts", bufs=1))
        work = ctx.enter_context(tc.tile_pool(name="work", bufs=1))
        keep = ctx.enter_context(tc.tile_pool(name="keep", bufs=1))
        res_pool = ctx.enter_context(tc.tile_pool(name="res", bufs=1))
        psum = ctx.enter_context(tc.tile_pool(name="psum", bufs=1, space="PSUM"))
        dram = ctx.enter_context(tc.tile_pool(name="dram", bufs=1, space="DRAM"))

        # ---------------- constants ----------------
        iota1024 = consts.tile([128, NSEG], f32)
        nc.gpsimd.iota(iota1024[:], [[1, NSEG]], channel_multiplier=0,
                       allow_small_or_imprecise_dtypes=True)
        # biota[j] = 32 - j  (for first-argmax via reduce_max)
        biota = consts.tile([128, C], f32)
        nc.gpsimd.iota(biota[:], [[-1, C]], base=C, channel_multiplier=0,
                       allow_small_or_imprecise_dtypes=True)
        ones128 = consts.tile([128, 1], f32)
        nc.vector.memset(ones128[:], 1.0)

        inbounce = dram.tile([ES, EF], f32, name="inbounce")
        outbounce = dram.tile([ES, EF], f32, name="outbounce", addr_space="Shared")

        # ---- batched input loads: 3 DMAs total (t first, x biggest last) ----
        tbig = keep.tile([128, KT, D * C], f32, name="tbig")
        nc.sync.dma_start(
            out=tbig[:], in_=targ[:, :].rearrange("(a p) f -> p a f", p=128))
        mkbig = keep.tile([128, KT, 1], f32, name="mkbig")
        nc.sync.dma_start(
            out=mkbig[:], in_=maskf[:, :].rearrange("(a p) f -> p a f", p=128))
        xbig = keep.tile([128, KT, FD], f32, name="xbig")
        nc.sync.dma_start(
            out=xbig[:], in_=feat[:, :].rearrange("(a p) f -> p a f", p=128))

        # split precision: one-hot block (tightness, bf16-insensitive) in bf16;
        # stats rows (diversity, cancellation-heavy) in fp32
        NST = 2 * D * C + 2   # 130 stats columns: [lq | p | ones | E]
        es_oh = [keep.tile([128, NSEG], bf16, name=f"esoh_{kc}")
                 for kc in range(KT)]
        es_st = [keep.tile([128, NST], f32, name=f"esst_{kc}")
                 for kc in range(KT)]
        ef_f32 = [keep.tile([128, EF], f32, name=f"eff_{kc}")
                  for kc in range(KT)]
        ef_b16 = [keep.tile([128, EF], bf16, name=f"efb_{kc}")
                  for kc in range(KT)]

        # ---- ACT phase 1: row sum-of-squares (Square table loads once) ----
        sqpack = keep.tile([128, KT], f32, name="sqpack")
        scrsq = keep.tile([128, FD], f32, name="scrsq")
        act_chain = []
        for kc in range(KT):
            act_chain.append(nc.scalar.activation(
                out=scrsq[:], in_=xbig[:, kc, :], func=Act.Square,
                accum_out=sqpack[:, kc:kc + 1]))
        # ---- ACT phase 2: one Sqrt for all chunks ----
        normpack = keep.tile([128, KT], f32, name="normpack")
        act_chain.append(nc.scalar.sqrt(normpack[:], sqpack[:]))
        nc.vector.tensor_scalar_max(out=normpack[:], in0=normpack[:],
                                    scalar1=1e-12)
        invpack = keep.tile([128, KT], f32, name="invpack")
        nc.vector.reciprocal(invpack[:], normpack[:])
        # minv = m * inv  (fold mask into the normalization scale)
        minvpack = keep.tile([128, KT], f32, name="minvpack")
        nc.vector.tensor_tensor(out=minvpack[:], in0=invpack[:],
                                in1=mkbig[:, :, 0], op=Alu.mult)

        # ---- targets chains (DVE) + Ln (ACT phase 3) ----
        # es_st columns: [0:64 lq | 64:128 p | 128 ones | 129 E]
        for kc in range(KT):
            st_t = es_st[kc]
            pt = st_t[:, D * C:2 * D * C]
            lqt = st_t[:, 0:D * C]
            t1 = work.tile([128, D * C], f32, name=f"t1_{kc}", tag=f"t1_{kc}")
            nc.vector.tensor_scalar_add(out=t1[:], in0=tbig[:, kc, :],
                                        scalar1=1e-10)
            ssum = work.tile([128, D], f32, name=f"ssum_{kc}", tag=f"ss_{kc}")
            nc.vector.reduce_sum(out=ssum[:],
                                 in_=t1[:].rearrange("p (d c) -> p d c", c=C),
                                 axis=Ax.X)
            invs = work.tile([128, D], f32, name=f"invs_{kc}", tag=f"iv_{kc}")
            nc.vector.reciprocal(invs[:], ssum[:])
            for d_ in range(D):
                nc.vector.tensor_scalar_mul(
                    out=pt[:, C * d_:C * (d_ + 1)],
                    in0=t1[:, C * d_:C * (d_ + 1)],
                    scalar1=invs[:, d_:d_ + 1])
            act_chain.append(nc.scalar.activation(out=lqt, in_=pt,
                                                  func=Act.Ln))
            scr64 = work.tile([128, D * C], f32, name=f"scr64_{kc}",
                              tag=f"s64_{kc}")
            nc.vector.tensor_tensor(out=scr64[:], in0=pt, in1=lqt,
                                    op=Alu.mult)
            nc.vector.reduce_sum(out=st_t[:, NST - 1:NST], in_=scr64[:],
                                 axis=Ax.X)
            nc.vector.memset(st_t[:, NST - 2:NST - 1], 1.0)

            # ---- first-argmax per dim, then code = cls0 + 32*cls1 ----
            cls = work.tile([128, D], f32, name=f"cls_{kc}", tag=f"cl_{kc}")
            for d_ in range(D):
                pch = pt[:, C * d_:C * (d_ + 1)]
                mx = work.tile([128, 1], f32, name=f"mx_{kc}_{d_}",
                               tag=f"mx_{kc}_{d_}")
                nc.vector.reduce_max(out=mx[:], in_=pch, axis=Ax.X)
                cand = work.tile([128, C], f32, name=f"cand_{kc}_{d_}",
                                 tag=f"cd_{kc}_{d_}")
                # (p == max) * (32 - idx); reduce_max -> 32 - first_argmax
                nc.vector.scalar_tensor_tensor(
                    out=cand[:], in0=pch, scalar=mx[:], in1=biota[:],
                    op0=Alu.is_equal, op1=Alu.mult)
                mq = work.tile([128, 1], f32, name=f"mq_{kc}_{d_}",
                               tag=f"mq_{kc}_{d_}")
                nc.vector.reduce_max(out=mq[:], in_=cand[:], axis=Ax.X)
                nc.vector.tensor_scalar(
                    out=cls[:, d_:d_ + 1], in0=mq[:], scalar1=-1.0,
                    scalar2=float(C), op0=Alu.mult, op1=Alu.add)
            code = work.tile([128, 1], f32, name=f"code_{kc}", tag=f"co_{kc}")
            nc.vector.tensor_scalar(
                out=code[:], in0=cls[:, 1:2], scalar1=float(C),
                scalar2=cls[:, 0:1], op0=Alu.mult, op1=Alu.add)
            # ---- one-hot (bf16 out: 0/1 exact) ----
            nc.vector.tensor_scalar(
                out=es_oh[kc][:], in0=iota1024[:], scalar1=code[:],
                scalar2=None, op0=Alu.is_equal)

        # ---- ext_feat = [x*(m*inv) | m | sq0*inv*minv] (ACT phase 4) ----
        for kc in range(KT):
            ef_t = ef_f32[kc]
            act_chain.append(nc.scalar.activation(
                out=ef_t[:, 0:FD], in_=xbig[:, kc, :], func=Act.Copy,
                scale=minvpack[:, kc:kc + 1]))
            nc.vector.tensor_copy(out=ef_t[:, FD:FD + 1], in_=mkbig[:, kc, :])
            nc.vector.tensor_scalar(out=ef_t[:, FD + 1:FD + 2],
                                    in0=sqpack[:, kc:kc + 1],
                                    scalar1=invpack[:, kc:kc + 1],
                                    scalar2=minvpack[:, kc:kc + 1],
                                    op0=Alu.mult, op1=Alu.mult)
            nc.vector.tensor_copy(out=ef_b16[kc][:], in_=ef_t[:])

        # keep ACT ops grouped by function (avoid act-table reload thrash)
        for a, b in zip(act_chain[1:], act_chain[:-1]):
            add_dep_helper(a.ins, b.ins, sync=False,
                           reason="act table grouping")

        # ---------------- the one big matmul ----------------
        resbig = keep.tile([128, NMT, EF], f32, name="resbig")
        for mt in range(NMT):
            mlo = mt * 128
            msz = min(128, ES - mlo)
            ps = psum.tile([msz, EF], f32, name=f"ps_{mt}", tag=f"ps_{mt % 6}")
            for kc in range(KT):
                if mt < 8:
                    lhsT = es_oh[kc][:, mlo:mlo + msz]
                    rhs = ef_b16[kc][:]
                else:
                    lhsT = es_st[kc][:, mlo - NSEG:mlo - NSEG + msz]
                    rhs = ef_f32[kc][:]
                nc.tensor.matmul(out=ps[:], lhsT=lhsT, rhs=rhs,
                                 start=(kc == 0), stop=(kc == KT - 1))
            nc.vector.tensor_copy(out=resbig[0:msz, mt, :], in_=ps[:])
        # three result stores on three different queues
        nc.sync.dma_start(
            out=inbounce[0:640, :].rearrange("(a p) f -> p a f", p=128),
            in_=resbig[:, 0:5, :])
        nc.gpsimd.dma_start(
            out=inbounce[640:1152, :].rearrange("(a p) f -> p a f", p=128),
            in_=resbig[:, 5:9, :])
        nc.scalar.dma_start(out=inbounce[1152:ES, :], in_=resbig[0:2, 9, :])

        # ---------------- single AllReduce ----------------
        nc.gpsimd.collective_compute(
            "AllReduce", mybir.AluOpType.add,
            replica_groups=[list(range(NCORES))],
            ins=[inbounce.opt()], outs=[outbounce.opt()])

        # ---------------- epilogue (redundant on every core) ----------------
        # two big strided loads: rows 0:512 and 512:1152 as [128, a, EF]
        big0 = keep.tile([128, 4, EF], f32, name="big0")
        nc.sync.dma_start(
            out=big0[:],
            in_=outbounce[0:512, :].rearrange("(a p) f -> p a f", p=128))
        big1 = keep.tile([128, 5, EF], f32, name="big1")
        nc.sync.dma_start(
            out=big1[:],
            in_=outbounce[512:1152, :].rearrange("(a p) f -> p a f", p=128))
        last2 = keep.tile([1, EF], f32, name="last2")
        nc.sync.dma_start(out=last2[:], in_=outbounce[ONES_COL:ONES_COL + 1, :])
        r1 = keep.tile([1, EF], f32, name="r1")
        nc.sync.dma_start(out=r1[:], in_=outbounce[E_COL:E_COL + 1, :])

        Z = keep.tile([128, 8], f32, name="Z")
        nc.vector.memset(Z[:], 0.0)
        nrmp = keep.tile([128, 8], f32, name="nrmp")
        cdp = keep.tile([128, 8], f32, name="cdp")
        scrA = keep.tile([128, 4, FD], f32, name="scrA")
        nc.scalar.activation(out=scrA[:], in_=big0[:, :, 0:FD],
                             func=Act.Square)
        nc.vector.reduce_sum(out=nrmp[:, 0:4], in_=scrA[:], axis=Ax.X)
        scrB = keep.tile([128, 4, FD], f32, name="scrB")
        nc.scalar.activation(out=scrB[:], in_=big1[:, 0:4, 0:FD],
                             func=Act.Square)
        nc.vector.reduce_sum(out=nrmp[:, 4:8], in_=scrB[:], axis=Ax.X)
        nc.vector.tensor_scalar_max(out=cdp[:, 0:4], in0=big0[:, :, FD],
                                    scalar1=1.0)
        nc.vector.tensor_scalar_max(out=cdp[:, 4:8], in0=big1[:, 0:4, FD],
                                    scalar1=1.0)
        rcdp = keep.tile([128, 8], f32, name="rcdp")
        nc.vector.reciprocal(rcdp[:], cdp[:])
        termp = keep.tile([128, 8], f32, name="termp")
        nc.vector.tensor_tensor(out=termp[:], in0=nrmp[:], in1=rcdp[:],
                                op=Alu.mult)
        nc.vector.reduce_sum(out=Z[:, 0:1], in_=termp[:], axis=Ax.X)

        # rows 1024:1152 live at big1[:, 4, :]: partitions 0:64 = U^T rows
        vt = keep.tile([64, EF], f32, name="vt")
        nc.sync.dma_start(out=vt[:], in_=outbounce[PCOL:PCOL + D * C, :])

        scrU = keep.tile([64, FD], f32, name="scrU")
        nc.vector.tensor_tensor(out=scrU[:], in0=big1[0:64, 4, 0:FD],
                                in1=vt[:, 0:FD], op=Alu.mult)
        nc.vector.reduce_sum(out=Z[0:64, 1:2], in_=scrU[:], axis=Ax.X)
        nc.vector.tensor_tensor(out=Z[0:64, 2:3], in0=vt[:, FD + 1:FD + 2],
                                in1=big1[0:64, 4, FD:FD + 1], op=Alu.mult)     # Psq*L
        nc.vector.tensor_tensor(out=Z[0:64, 3:4], in0=vt[:, FD:FD + 1],
                                in1=big1[0:64, 4, FD + 1:FD + 2], op=Alu.mult)  # Pbar*Lsq
        scrF = keep.tile([1, FD], f32, name="scrF")
        nc.vector.tensor_tensor(out=scrF[:], in0=last2[:, 0:FD],
                                in1=r1[:, 0:FD], op=Alu.mult)
        nc.vector.reduce_sum(out=Z[0:1, 4:5], in_=scrF[:], axis=Ax.X)  # Fe.F

        zred = psum.tile([1, 8], f32, name="zred", tag="zred")
        nc.tensor.matmul(out=zred[:], lhsT=ones128[:], rhs=Z[:],
                         start=True, stop=True)
        zs = keep.tile([1, 8], f32, name="zs")
        nc.vector.tensor_copy(out=zs[:], in_=zred[:])

        # scalars: M=last2[256], a=last2[257], e=r1[256], se=r1[257]
        Mv = last2[0:1, FD:FD + 1]
        av = last2[0:1, FD + 1:FD + 2]
        ev = r1[0:1, FD:FD + 1]
        sev = r1[0:1, FD + 1:FD + 2]
        s_center = zs[0:1, 0:1]
        uv = zs[0:1, 1:2]
        psql = zs[0:1, 2:3]
        pbarlsq = zs[0:1, 3:4]
        fef = zs[0:1, 4:5]

        fin = keep.tile([1, 16], f32, name="fin")
        t_ = lambda i: fin[0:1, i:i + 1]
        # f0 = se*M ; f1 = a*e ; f2 = f0+f1
        nc.vector.tensor_tensor(out=t_(8), in0=sev, in1=Mv, op=Alu.mult)
        nc.vector.tensor_tensor(out=t_(9), in0=av, in1=ev, op=Alu.mult)
        nc.vector.tensor_tensor(out=t_(10), in0=t_(8), in1=t_(9), op=Alu.add)
        # f3 = -2*fef + f2
        nc.vector.tensor_scalar(out=t_(11), in0=fef, scalar1=-2.0,
                                scalar2=t_(10), op0=Alu.mult, op1=Alu.add)
        # f4 = f3 - psql ; f5 = f4 - pbarlsq
        nc.vector.tensor_tensor(out=t_(12), in0=t_(11), in1=psql, op=Alu.subtract)
        nc.vector.tensor_tensor(out=t_(13), in0=t_(12), in1=pbarlsq, op=Alu.subtract)
        # SD = 2*uv + f5
        nc.vector.tensor_scalar(out=t_(14), in0=uv, scalar1=2.0,
                                scalar2=t_(13), op0=Alu.mult, op1=Alu.add)
        # md = M*(M-1) ; rmd = 1/md ; div = SD*rmd*(-1/D)
        nc.vector.tensor_scalar(out=t_(15), in0=Mv, scalar1=-1.0,
                                scalar2=Mv, op0=Alu.add, op1=Alu.mult)
        nc.vector.reciprocal(t_(15), t_(15))
        nc.vector.tensor_tensor(out=t_(1), in0=t_(14), in1=t_(15), op=Alu.mult)
        nc.vector.tensor_scalar_mul(out=t_(1), in0=t_(1), scalar1=-1.0 / D)
        # tight = (a - s_center)/M
        nc.vector.tensor_tensor(out=t_(7), in0=av, in1=s_center, op=Alu.subtract)
        nc.vector.reciprocal(t_(6), Mv)
        nc.vector.tensor_tensor(out=t_(2), in0=t_(7), in1=t_(6), op=Alu.mult)
        # total = 0.1*div + 0.1*tight
        nc.vector.tensor_tensor(out=t_(0), in0=t_(1), in1=t_(2), op=Alu.add)
        nc.vector.tensor_scalar_mul(out=t_(0), in0=t_(0), scalar1=0.1)
        # debug slots
        nc.vector.tensor_copy(out=t_(3), in_=Mv)
        nc.vector.tensor_copy(out=t_(4), in_=av)
        nc.vector.tensor_copy(out=t_(5), in_=sev)

        nc.sync.dma_start(out=outd[None, :], in_=fin[0:1, 0:8])

    nc.finalize()
    return nc


def _get_compiled():
    if "nc" not in _compiled:
        _compiled["nc"] = _build_bass()
    return _compiled["nc"]


def _make_in_maps(features, targets, mask):
    features = np.ascontiguousarray(np.asarray(features, dtype=np.float32))
    targets = np.ascontiguousarray(np.asarray(targets, dtype=np.float32))
    maskf = np.asarray(mask).astype(np.float32).reshape(B, 1)
    in_maps = []
    for i in range(NCORES):
        sl = slice(i * RB, (i + 1) * RB)
        in_maps.append({
            "features": features[sl],
            "targets": targets[sl],
            "maskf": np.ascontiguousarray(maskf[sl]),
        })
    return in_maps


def kernel(features, targets, mask):
    from concourse.bass_utils import run_bass_kernel_spmd

    nc = _get_compiled()
    in_maps = _make_in_maps(features, targets, mask)
    res = run_bass_kernel_spmd(nc, in_maps, list(range(NCORES)))
    out = res.results[0]["out"]
    total = np.float32(out[0])
    diversity = np.float32(out[1])
    tightness = np.float32(out[2])
    return total, diversity, tightness
